# Initial kernel scaffold
#
"""Trainium2 Bass kernel for nn_ConditioningEncoder.

Pipeline per position: f0/dur scalar MLPs + phone/midi embedding lookups
-> concat -> Linear(320,256) -> LayerNorm -> ReLU -> Linear(256,256).

Strategy (data parallel over 8 cores, 8192 positions each):
- Host folds the small linears: the f0/dur second-layer weights and the
  embedding tables are pre-multiplied by the corresponding row-blocks of
  proj_w1, so the device only does:
    h = relu(f0*w1+b1 | dur*...) @ fdW  +  onehot(phone) @ phW  +  onehot(midi) @ miW
  with all biases folded into the phone table rows.
- Embedding gathers are one-hot matmuls on the PE (tables are tiny).
- LayerNorm stats via bn_stats/bn_aggr; normalize+ReLU fused into one
  scalar-engine activation (per-partition scale=rstd, bias=-mu*rstd).
- y is transposed for the second matmul via DMA xbar block transposes.
- Output staged in SBUF and written back in 1MB DMAs.
- All constants ride in one bf16 + one f32 tensor (2 DMAs) to keep the
  per-instruction semaphore-wait fan-in low.
"""

import numpy as np
import ml_dtypes
from contextlib import ExitStack

import concourse.bass as bass
import concourse.mybir as mybir
import concourse.tile as tile
from concourse import bacc
from concourse.bass_utils import run_bass_kernel_spmd

BF16 = mybir.dt.bfloat16
F32 = mybir.dt.float32
NCORES = 8
B, T, COND = 16, 4096, 256
NPOS = B * T                     # 65536
PER_CORE = NPOS // NCORES        # 8192
NTILES = PER_CORE // 128         # 64 tiles of 128 positions
SUPER = 4                        # tiles per super-tile (512 positions)
OUT_GROUP = 8                    # tiles per output DMA (1MB)
EPS = 1e-5
BFC_COLS = 1728

_cache = {}


def _build_program(apply_gb: bool):
    per_core = PER_CORE
    ntiles = per_core // 128
    nsuper = ntiles // SUPER

    nc = bacc.Bacc("TRN2", target_bir_lowering=False, debug=False)

    # ---- DRAM I/O ----
    d_fd = nc.dram_tensor("fd", [2, per_core], BF16, kind="ExternalInput")
    d_ph = nc.dram_tensor("ph", [1, per_core], BF16, kind="ExternalInput")
    d_mi = nc.dram_tensor("mi", [1, per_core], BF16, kind="ExternalInput")
    d_bfc = nc.dram_tensor("bfc", [128, BFC_COLS], BF16, kind="ExternalInput")
    d_f32c = nc.dram_tensor("f32c", [128, 2], F32, kind="ExternalInput")
    if apply_gb:
        d_gbc = nc.dram_tensor("g_bc", [128, 256], F32, kind="ExternalInput")
        d_bbc = nc.dram_tensor("b_bc", [128, 256], F32, kind="ExternalInput")
    d_out = nc.dram_tensor("out", [per_core, 256], F32, kind="ExternalOutput")

    with tile.TileContext(nc) as tc, ExitStack() as ctx:
        singles = ctx.enter_context(tc.tile_pool(name="singles", bufs=1))
        sb_oh = ctx.enter_context(tc.tile_pool(name="oh", bufs=2))
        sb_fdh = ctx.enter_context(tc.tile_pool(name="fdh", bufs=2))
        sb_small = ctx.enter_context(tc.tile_pool(name="small", bufs=3))
        sb_y = ctx.enter_context(tc.tile_pool(name="y", bufs=3))
        sb_yt = ctx.enter_context(tc.tile_pool(name="yt", bufs=3))
        sb_out = ctx.enter_context(tc.tile_pool(name="ostage", bufs=2))
        pp_bc = ctx.enter_context(tc.tile_pool(name="pbc", bufs=2, space="PSUM"))
        pp_fd = ctx.enter_context(tc.tile_pool(name="pfd", bufs=1, space="PSUM"))
        pp_h = ctx.enter_context(tc.tile_pool(name="ph_", bufs=2, space="PSUM"))
        pp_o = ctx.enter_context(tc.tile_pool(name="po", bufs=2, space="PSUM"))

        # ---- load inputs/constants into SBUF (few DMAs; low sem fan-in) ----
        s_fd = singles.tile([2, per_core], BF16, tag="c_fd")
        nc.gpsimd.dma_start(out=s_fd[:], in_=d_fd[:])
        s_ph = singles.tile([1, per_core], BF16, tag="c_ph")
        nc.gpsimd.dma_start(out=s_ph[:], in_=d_ph[:])
        s_mi = singles.tile([1, per_core], BF16, tag="c_mi")
        nc.gpsimd.dma_start(out=s_mi[:], in_=d_mi[:])
        s_bfc = singles.tile([128, BFC_COLS], BF16, tag="c_bfc")
        nc.gpsimd.dma_start(out=s_bfc[:], in_=d_bfc[:])
        s_f32c = singles.tile([128, 2], F32, tag="c_f32c")
        nc.gpsimd.dma_start(out=s_f32c[:], in_=d_f32c[:])
        if apply_gb:
            s_gbc = singles.tile([128, 256], F32, tag="c_gbc")
            nc.gpsimd.dma_start(out=s_gbc[:], in_=d_gbc[:])
            s_bbc = singles.tile([128, 256], F32, tag="c_bbc")
            nc.gpsimd.dma_start(out=s_bbc[:], in_=d_bbc[:])
        s_eps = singles.tile([128, 1], F32, tag="eps")
        nc.vector.memset(s_eps, EPS)

        # views into the packed constant tile
        s_fdw = s_bfc[0:64, 0:256]
        s_phw = s_bfc[:, 256:512]
        s_miw = s_bfc[:, 512:768]
        s_w2a = s_bfc[:, 768:1024]
        s_w2b = s_bfc[:, 1024:1280]
        s_b2 = s_bfc[0:1, 1280:1536]
        s_ones = s_bfc[0:1, 1536:1664]
        s_w1 = s_bfc[0:2, 1664:1728]
        s_iota = s_f32c[:, 0:1]
        s_b1 = s_f32c[0:64, 1:2]

        out_r = d_out.ap().rearrange("(gi t p) c -> gi t p c", t=OUT_GROUP, p=128)

        ostage = None
        for st in range(nsuper):
            W = SUPER * 128  # 512
            sl = slice(st * W, (st + 1) * W)

            # broadcast phone/midi values across partitions (outer product w/ ones)
            bc_ph = pp_bc.tile([128, W], F32, tag="bc")
            nc.tensor.matmul(bc_ph[:], s_ones, s_ph[:, sl], start=True, stop=True)
            oh_ph = sb_oh.tile([128, W], BF16, tag="ohp")
            nc.vector.tensor_scalar(
                out=oh_ph[:], in0=bc_ph[:], scalar1=s_iota, scalar2=None,
                op0=mybir.AluOpType.is_equal)
            bc_mi = pp_bc.tile([128, W], F32, tag="bc")
            nc.tensor.matmul(bc_mi[:], s_ones, s_mi[:, sl], start=True, stop=True)
            oh_mi = sb_oh.tile([128, W], BF16, tag="ohm")
            nc.vector.tensor_scalar(
                out=oh_mi[:], in0=bc_mi[:], scalar1=s_iota, scalar2=None,
                op0=mybir.AluOpType.is_equal)

            # f0/dur hidden: outer product then relu+bias
            fdp = pp_fd.tile([64, W], F32, tag="fdp")
            nc.tensor.matmul(fdp[:], s_w1, s_fd[:, sl], start=True, stop=True)
            fdh = sb_fdh.tile([64, W], BF16, tag="fdh")
            nc.scalar.activation(
                out=fdh[:], in_=fdp[:], func=mybir.ActivationFunctionType.Relu,
                bias=s_b1, scale=1.0)

            for pair in range(SUPER // 2):
                hps = pp_h.tile([128, 2, 256], F32, tag="hps")
                for tt in range(2):
                    lo = (pair * 2 + tt) * 128
                    nc.tensor.matmul(hps[:, tt, :], fdh[:, lo:lo + 128], s_fdw,
                                     start=True, stop=False)
                    nc.tensor.matmul(hps[:, tt, :], oh_ph[:, lo:lo + 128], s_phw,
                                     start=False, stop=False)
                    nc.tensor.matmul(hps[:, tt, :], oh_mi[:, lo:lo + 128], s_miw,
                                     start=False, stop=True)
                stats = sb_small.tile([128, 2, 6], F32, tag="stats")
                mv = sb_small.tile([128, 2, 2], F32, tag="mv")
                for tt in range(2):
                    nc.vector.bn_stats(out=stats[:, tt, :], in_=hps[:, tt, :])
                    nc.vector.bn_aggr(out=mv[:, tt, :], in_=stats[:, tt, :])
                # rstd = 1/sqrt(var+eps); nmr = -mu*rstd   (both tiles at once)
                rstd = sb_small.tile([128, 2], F32, tag="rstd")
                nc.scalar.activation(
                    out=rstd[:], in_=mv[:, :, 1], func=mybir.ActivationFunctionType.Sqrt,
                    bias=s_eps[:], scale=1.0)
                nc.vector.reciprocal(out=rstd[:], in_=rstd[:])
                negmu = sb_small.tile([128, 2], F32, tag="negmu")
                nc.vector.tensor_scalar(
                    out=negmu[:], in0=mv[:, :, 0], scalar1=-1.0, scalar2=None,
                    op0=mybir.AluOpType.mult)
                nmr = sb_small.tile([128, 2], F32, tag="nmr")
                nc.vector.tensor_mul(out=nmr[:], in0=negmu[:], in1=rstd[:])

                for tt in range(2):
                    gt = st * SUPER + pair * 2 + tt     # global tile idx
                    og = gt % OUT_GROUP
                    if og == 0:
                        ostage = sb_out.tile([128, OUT_GROUP, 256], F32, tag="ost")

                    y = sb_y.tile([128, 256], BF16, tag="y")
                    if not apply_gb:
                        nc.scalar.activation(
                            out=y[:], in_=hps[:, tt, :],
                            func=mybir.ActivationFunctionType.Relu,
                            bias=nmr[:, tt:tt + 1], scale=rstd[:, tt:tt + 1])
                    else:
                        yn = sb_y.tile([128, 256], F32, tag="yn")
                        nc.scalar.activation(
                            out=yn[:], in_=hps[:, tt, :],
                            func=mybir.ActivationFunctionType.Identity,
                            bias=nmr[:, tt:tt + 1], scale=rstd[:, tt:tt + 1])
                        nc.vector.tensor_mul(out=yn[:], in0=yn[:], in1=s_gbc[:])
                        nc.vector.tensor_add(out=yn[:], in0=yn[:], in1=s_bbc[:])
                        nc.vector.tensor_scalar(
                            out=y[:], in0=yn[:], scalar1=0.0, scalar2=None,
                            op0=mybir.AluOpType.max)

                    yta = sb_yt.tile([128, 128], BF16, tag="yta")
                    ytb = sb_yt.tile([128, 128], BF16, tag="ytb")
                    nc.sync.dma_start_transpose(yta[:], y[:, 0:128])
                    nc.sync.dma_start_transpose(ytb[:], y[:, 128:256])

                    # bias matmul first: its inputs are constants, so the
                    # accumulation group's PSUM waits land on a wait-free op
                    ops = pp_o.tile([128, 256], F32, tag="ops")
                    nc.tensor.matmul(ops[:], s_ones, s_b2, start=True, stop=False)
                    nc.tensor.matmul(ops[:], yta[:], s_w2a, start=False, stop=False)
                    nc.tensor.matmul(ops[:], ytb[:], s_w2b, start=False, stop=True)

                    if tt == 0:
                        nc.vector.tensor_copy(out=ostage[:, og, :], in_=ops[:])
                    else:
                        nc.scalar.copy(out=ostage[:, og, :], in_=ops[:])

                    if og == OUT_GROUP - 1:
                        gi = gt // OUT_GROUP
                        nc.gpsimd.dma_start(
                            out=out_r[gi].rearrange("t p c -> p t c"),
                            in_=ostage[:])
    nc.compile()
    return nc


def _prep(inputs):
    """Host-side constant folding. Returns (apply_gb, per-core input maps)."""
    f0 = np.ascontiguousarray(inputs["f0"], dtype=np.float32)
    dur = np.ascontiguousarray(inputs["duration"], dtype=np.float32)
    phone = np.asarray(inputs["phone"])
    midi = np.asarray(inputs["midi"])

    w1f, b1f = np.asarray(inputs["f0_w1"], np.float32), np.asarray(inputs["f0_b1"], np.float32)
    w2f, b2f = np.asarray(inputs["f0_w2"], np.float32), np.asarray(inputs["f0_b2"], np.float32)
    w1d, b1d = np.asarray(inputs["dur_w1"], np.float32), np.asarray(inputs["dur_b1"], np.float32)
    w2d, b2d = np.asarray(inputs["dur_w2"], np.float32), np.asarray(inputs["dur_b2"], np.float32)
    pht = np.asarray(inputs["phone_table"], np.float32)
    mit = np.asarray(inputs["midi_table"], np.float32)
    W1, pb1 = np.asarray(inputs["proj_w1"], np.float32), np.asarray(inputs["proj_b1"], np.float32)
    ln_g, ln_b = np.asarray(inputs["ln_g"], np.float32), np.asarray(inputs["ln_b"], np.float32)
    W2, pb2 = np.asarray(inputs["proj_w2"], np.float32), np.asarray(inputs["proj_b2"], np.float32)

    W1_f0, W1_ph = W1[0:64], W1[64:192]
    W1_mi, W1_du = W1[192:256], W1[256:320]

    fdw = np.vstack([w2f @ W1_f0, w2d @ W1_du])                      # [64,256]
    bias_total = pb1 + b2f @ W1_f0 + b2d @ W1_du                     # [256]
    phw = np.zeros((128, 256), np.float32)
    phw[: pht.shape[0]] = pht @ W1_ph + bias_total
    miw = mit @ W1_mi                                                # [128,256]

    fold_g = bool((ln_g > 0).all() and (ln_b == 0).all())
    apply_gb = not fold_g
    W2e = (ln_g[:, None] * W2) if fold_g else W2

    bf = ml_dtypes.bfloat16
    bfc = np.zeros((128, BFC_COLS), np.float32)
    bfc[0:64, 0:256] = fdw
    bfc[:, 256:512] = phw
    bfc[:, 512:768] = miw
    bfc[:, 768:1024] = W2e[0:128]
    bfc[:, 1024:1280] = W2e[128:256]
    bfc[0, 1280:1536] = pb2
    bfc[0, 1536:1664] = 1.0
    bfc[0, 1664:1696] = w1f[0]
    bfc[1, 1696:1728] = w1d[0]
    f32c = np.zeros((128, 2), np.float32)
    f32c[:, 0] = np.arange(128)
    f32c[0:64, 1] = np.concatenate([b1f, b1d])

    per_core = f0.size // NCORES
    consts = {"bfc": bfc.astype(bf), "f32c": f32c}
    if apply_gb:
        consts["g_bc"] = np.broadcast_to(ln_g, (128, 256)).astype(np.float32).copy()
        consts["b_bc"] = np.broadcast_to(ln_b, (128, 256)).astype(np.float32).copy()

    f0v = f0.reshape(NCORES, per_core)
    durv = dur.reshape(NCORES, per_core)
    phv = phone.reshape(NCORES, per_core).astype(bf)
    miv = midi.reshape(NCORES, per_core).astype(bf)

    in_maps = []
    for c in range(NCORES):
        m = dict(consts)
        m["fd"] = np.stack([f0v[c], durv[c]]).astype(bf)
        m["ph"] = phv[c].reshape(1, per_core)
        m["mi"] = miv[c].reshape(1, per_core)
        in_maps.append(m)
    return apply_gb, in_maps


def kernel(**inputs) -> np.ndarray:
    apply_gb, in_maps = _prep(inputs)
    if apply_gb not in _cache:
        _cache[apply_gb] = _build_program(apply_gb)
    nc = _cache[apply_gb]
    res = run_bass_kernel_spmd(nc, in_maps, core_ids=list(range(NCORES)))
    out = np.concatenate([r["out"] for r in res.results], axis=0)
    return out.reshape(B, T, COND).astype(np.float32)



# revision 27
# speedup vs baseline: 1.1675x; 1.1675x over previous
"""Trainium2 Bass kernel for nn_ConditioningEncoder.

Pipeline per position: f0/dur scalar MLPs + phone/midi embedding lookups
-> concat -> Linear(320,256) -> LayerNorm -> ReLU -> Linear(256,256).

Fast path (ln_g > 0, ln_b == 0 -- the actual problem instance):
- Host folds the small linears AND the LayerNorm mean-subtraction into
  the first projection: centering is linear, so W1' = W1 @ (I - J/256)
  makes the device's first-layer output h_c exactly zero-mean.  Then
  LN+ReLU reduces to  y = rstd * relu(h_c)  with rstd = 1/sqrt(mean
  (h_c^2)+eps), and g>0 folds into W2.
- Layer 1 runs COND-MAJOR: stationary weight blocks [K,128] stream the
  positions, producing h_c^T [2x128cond, W pos] straight into PSUM.
  This needs only 6 matmuls per 512-position supertile and -- key --
  makes h_c^T itself the stationary operand layer 2 needs, so the
  per-tile DMA transposes of the old design disappear entirely.
- Embedding gathers are one-hot matmuls; the one-hot operands are built
  by a GpSimd partition_broadcast + one DVE is_equal against iota
  (no PE broadcast matmuls).
- sum(h_c^2) via a ones-column matmul over both cond halves; rstd is
  broadcast back to all partitions with a K=1 ones-row matmul and
  applied to y in cond-major on GpSimd.
- Layer 2: lhsT = y^T slices (data stationary), rhs = W2 halves; the
  PSUM drain is a single DVE tensor_tensor add of the pre-broadcast
  output bias into the staged output buffer; 1MB output DMAs.

Fallback path (general ln_g/ln_b): previous-generation program (DMA
transposes + bn_stats), kept verbatim for correctness.
"""

import numpy as np
import ml_dtypes
from contextlib import ExitStack

import concourse.bass as bass
import concourse.mybir as mybir
import concourse.tile as tile
from concourse import bacc
from concourse.bass_utils import run_bass_kernel_spmd

BF16 = mybir.dt.bfloat16
F32 = mybir.dt.float32
NCORES = 8
B, T, COND = 16, 4096, 256
NPOS = B * T                     # 65536
PER_CORE = NPOS // NCORES        # 8192
NTILES = PER_CORE // 128         # 64 tiles of 128 positions
OUT_GROUP = 8                    # tiles per output DMA (1MB)
EPS = 1e-5
BFC_COLS = 1728

# ---- v2 (fast path) constants ----
W2SUP = 512                      # positions per supertile
NSUPER = PER_CORE // W2SUP       # 16
TPS = W2SUP // 128               # 4 tiles per supertile
V2_BCOLS = 1536

_cache = {}


def _build_program_v2(add_pb2: bool):
    per_core = PER_CORE
    W = W2SUP

    nc = bacc.Bacc("TRN2", target_bir_lowering=False, debug=False)

    d_fd = nc.dram_tensor("fd", [1, 2 * per_core], F32, kind="ExternalInput")
    d_phmi = nc.dram_tensor("phmi", [1, 2 * per_core], BF16, kind="ExternalInput")
    d_bfc = nc.dram_tensor("bfc", [128, V2_BCOLS], BF16, kind="ExternalInput")
    d_f32c = nc.dram_tensor("f32c", [128, 4], F32, kind="ExternalInput")
    d_pb2 = nc.dram_tensor("pb2bc", [128, 256], F32, kind="ExternalInput")
    d_out = nc.dram_tensor("out", [per_core, 256], F32, kind="ExternalOutput")

    with tile.TileContext(nc) as tc, ExitStack() as ctx:
        singles = ctx.enter_context(tc.tile_pool(name="singles", bufs=1))
        p_bc = ctx.enter_context(tc.tile_pool(name="bc", bufs=3))
        p_oh = ctx.enter_context(tc.tile_pool(name="oh", bufs=3))
        p_fd = ctx.enter_context(tc.tile_pool(name="fd", bufs=3))
        p_h = ctx.enter_context(tc.tile_pool(name="hsb", bufs=3))
        p_y = ctx.enter_context(tc.tile_pool(name="y", bufs=3))
        p_sm = ctx.enter_context(tc.tile_pool(name="sm", bufs=3))
        p_ost = ctx.enter_context(tc.tile_pool(name="ost", bufs=2))
        pp_h = ctx.enter_context(tc.tile_pool(name="ph_", bufs=2, space="PSUM"))
        pp_v = ctx.enter_context(tc.tile_pool(name="pv", bufs=2, space="PSUM"))
        pp_o = ctx.enter_context(tc.tile_pool(name="po", bufs=2, space="PSUM"))

        # ---- load constants (inputs stream in broadcast per-super) ----
        s_bfc = singles.tile([128, V2_BCOLS], BF16, tag="c_bfc")
        nc.gpsimd.dma_start(out=s_bfc[:], in_=d_bfc[:])
        s_f32c = singles.tile([128, 4], F32, tag="c_f32c")
        nc.gpsimd.dma_start(out=s_f32c[:], in_=d_f32c[:])
        if add_pb2:
            s_pb2 = singles.tile([128, 256], F32, tag="c_pb2")
            nc.gpsimd.dma_start(out=s_pb2[:], in_=d_pb2[:])

        s_fdw = s_bfc[0:64, 0:256]       # [64, 2x128] cond-major fd weights
        s_phw = s_bfc[:, 256:512]        # [128, 2x128]
        s_miw = s_bfc[:, 512:768]
        s_w2a = s_bfc[:, 768:1024]       # W2 rows 0:128 -> [128, 256]
        s_w2b = s_bfc[:, 1024:1280]
        s_ones_blk = s_bfc[:, 1280:1408]     # [128, 128] of 1.0
        s_iota = s_f32c[:, 0:1]
        s_w1 = s_f32c[0:64, 1:2]
        s_b1 = s_f32c[0:64, 2:3]
        s_eps = s_f32c[:, 3:4]               # [128, 1] of EPS

        phmi_r = d_phmi.ap().rearrange("p (c n) -> p c n", c=2)  # [1, 2, per_core]
        fd_r = d_fd.ap().rearrange("p (c n) -> p c n", c=2)      # [1, 2, per_core]
        out_r = d_out.ap().rearrange("(gi t p) c -> gi t p c", t=OUT_GROUP, p=128)

        ostage = None
        for st in range(NSUPER):
            sl = slice(st * W, (st + 1) * W)

            # one-hot build: DMA-broadcast ph|mi to all partitions (stride-0
            # source), compare against per-partition iota
            bc = p_bc.tile([128, 2, W], BF16, tag="bc")
            nc.sync.dma_start(out=bc[:], in_=phmi_r[:, :, sl].broadcast_to([128, 2, W]))
            oh = p_oh.tile([128, 2, W], BF16, tag="oh")
            nc.gpsimd.tensor_scalar(
                out=oh[:], in0=bc[:], scalar1=s_iota, scalar2=None,
                op0=mybir.AluOpType.is_equal)

            # f0/dur hidden: broadcast raw scalars, then per-partition w1/b1
            fdbc = p_fd.tile([64, 2, W], F32, tag="fdbc")
            nc.sync.dma_start(out=fdbc[:], in_=fd_r[:, :, sl].broadcast_to([64, 2, W]))
            fdh = p_fd.tile([64, W], BF16, tag="fdh")
            nc.scalar.activation(
                out=fdh[0:32, :], in_=fdbc[0:32, 0, :],
                func=mybir.ActivationFunctionType.Relu,
                bias=s_b1[0:32, :], scale=s_w1[0:32, :])
            nc.scalar.activation(
                out=fdh[32:64, :], in_=fdbc[32:64, 1, :],
                func=mybir.ActivationFunctionType.Relu,
                bias=s_b1[32:64, :], scale=s_w1[32:64, :])

            # layer 1, cond-major: h_c^T [128, 2(half), W]
            hps = pp_h.tile([128, 2, W], F32, tag="hps")
            for hf in range(2):
                cs = slice(hf * 128, (hf + 1) * 128)
                nc.tensor.matmul(hps[:, hf, :], s_fdw[:, cs], fdh[:],
                                 start=True, stop=False)
                nc.tensor.matmul(hps[:, hf, :], s_phw[:, cs], oh[:, 0, :],
                                 start=False, stop=False)
                nc.tensor.matmul(hps[:, hf, :], s_miw[:, cs], oh[:, 1, :],
                                 start=False, stop=True)

            # square straight from PSUM (no staging copy)
            hsq = p_h.tile([128, 2, W], BF16, tag="hsq")
            nc.scalar.activation(
                out=hsq[:], in_=hps[:], func=mybir.ActivationFunctionType.Square)

            # sum h^2 over both cond halves with an all-ones stationary
            # block: every output partition gets the same row of sums
            var_ps = pp_v.tile([128, W], F32, tag="var")
            nc.tensor.matmul(var_ps[:], s_ones_blk, hsq[:, 0, :],
                             start=True, stop=False)
            nc.tensor.matmul(var_ps[:], s_ones_blk, hsq[:, 1, :],
                             start=False, stop=True)
            # rstd = 1/sqrt(sumsq/256 + eps), full-width on all partitions
            sd = p_sm.tile([128, W], F32, tag="sd")
            nc.scalar.activation(
                out=sd[:], in_=var_ps[:], func=mybir.ActivationFunctionType.Sqrt,
                bias=s_eps, scale=1.0 / COND)
            rbc = p_sm.tile([128, W], F32, tag="rbcsb")
            nc.vector.reciprocal_approx_fast(out=rbc[:], in_=sd[:])

            # y_n = relu(h) * rstd fused in one DVE op straight from PSUM
            yn = p_y.tile([128, 2, W], BF16, tag="yn")
            nc.vector.scalar_tensor_tensor(
                out=yn[:], in0=hps[:], scalar=0.0,
                in1=rbc.rearrange("p (c n) -> p c n", c=1).broadcast_to([128, 2, W]),
                op0=mybir.AluOpType.max, op1=mybir.AluOpType.mult)

            # layer 2 per 128-pos tile: lhsT = y_n^T slices, rhs = W2 halves
            for t in range(TPS):
                gt = st * TPS + t
                og = gt % OUT_GROUP
                if og == 0:
                    ostage = p_ost.tile([128, OUT_GROUP, 256], F32, tag="ost")
                ch = slice(t * 128, (t + 1) * 128)
                ops = pp_o.tile([128, 256], F32, tag="ops")
                nc.tensor.matmul(ops[:], yn[:, 0, ch], s_w2a,
                                 start=True, stop=False)
                nc.tensor.matmul(ops[:], yn[:, 1, ch], s_w2b,
                                 start=False, stop=True)
                if add_pb2:
                    # drain + output bias in one DVE op
                    nc.vector.tensor_add(out=ostage[:, og, :], in0=ops[:], in1=s_pb2[:])
                elif t % 2 == 0:
                    nc.scalar.copy(out=ostage[:, og, :], in_=ops[:])
                else:
                    nc.vector.tensor_copy(out=ostage[:, og, :], in_=ops[:])

                if og == OUT_GROUP - 1:
                    gi = gt // OUT_GROUP
                    nc.gpsimd.dma_start(
                        out=out_r[gi].rearrange("t p c -> p t c"),
                        in_=ostage[:])
    nc.compile()
    return nc


def _prep_v2(inputs):
    f0 = np.ascontiguousarray(inputs["f0"], dtype=np.float32)
    dur = np.ascontiguousarray(inputs["duration"], dtype=np.float32)
    phone = np.asarray(inputs["phone"])
    midi = np.asarray(inputs["midi"])

    w1f, b1f = np.asarray(inputs["f0_w1"], np.float32), np.asarray(inputs["f0_b1"], np.float32)
    w2f, b2f = np.asarray(inputs["f0_w2"], np.float32), np.asarray(inputs["f0_b2"], np.float32)
    w1d, b1d = np.asarray(inputs["dur_w1"], np.float32), np.asarray(inputs["dur_b1"], np.float32)
    w2d, b2d = np.asarray(inputs["dur_w2"], np.float32), np.asarray(inputs["dur_b2"], np.float32)
    pht = np.asarray(inputs["phone_table"], np.float32)
    mit = np.asarray(inputs["midi_table"], np.float32)
    W1, pb1 = np.asarray(inputs["proj_w1"], np.float32), np.asarray(inputs["proj_b1"], np.float32)
    ln_g = np.asarray(inputs["ln_g"], np.float32)
    W2, pb2 = np.asarray(inputs["proj_w2"], np.float32), np.asarray(inputs["proj_b2"], np.float32)

    W1_f0, W1_ph = W1[0:64], W1[64:192]
    W1_mi, W1_du = W1[192:256], W1[256:320]

    # fold LN mean-centering (linear!) into the first projection
    C = np.eye(COND, dtype=np.float64) - 1.0 / COND
    fdw = (np.vstack([w2f @ W1_f0, w2d @ W1_du]) @ C).astype(np.float32)
    bias_total = ((pb1 + b2f @ W1_f0 + b2d @ W1_du) @ C).astype(np.float32)
    phw = np.zeros((128, COND), np.float32)
    phw[: pht.shape[0]] = ((pht @ W1_ph) @ C).astype(np.float32) + bias_total
    miw = ((mit @ W1_mi) @ C).astype(np.float32)

    W2e = ln_g[:, None] * W2

    bf = ml_dtypes.bfloat16
    bfc = np.zeros((128, V2_BCOLS), np.float32)
    bfc[0:64, 0:256] = fdw
    bfc[:, 256:512] = phw
    bfc[:, 512:768] = miw
    bfc[:, 768:1024] = W2e[0:128]
    bfc[:, 1024:1280] = W2e[128:256]
    bfc[:, 1280:1408] = 1.0
    f32c = np.zeros((128, 4), np.float32)
    f32c[:, 0] = np.arange(128)
    f32c[0:32, 1] = w1f[0]
    f32c[32:64, 1] = w1d[0]
    f32c[0:64, 2] = np.concatenate([b1f, b1d])
    f32c[:, 3] = EPS
    pb2bc = np.broadcast_to(pb2, (128, 256)).astype(np.float32).copy()

    consts = {"bfc": bfc.astype(bf), "f32c": f32c, "pb2bc": pb2bc}

    f0v = f0.reshape(NCORES, PER_CORE)
    durv = dur.reshape(NCORES, PER_CORE)
    phv = phone.reshape(NCORES, PER_CORE).astype(bf)
    miv = midi.reshape(NCORES, PER_CORE).astype(bf)

    in_maps = []
    for c in range(NCORES):
        m = dict(consts)
        m["fd"] = np.concatenate([f0v[c], durv[c]]).reshape(1, 2 * PER_CORE)
        m["phmi"] = np.concatenate([phv[c], miv[c]]).reshape(1, 2 * PER_CORE)
        in_maps.append(m)
    return in_maps


# ======================= fallback (general ln_g/ln_b) =======================

SUPER = 4
BFC_COLS = 1728


def _build_program(apply_gb: bool):
    per_core = PER_CORE
    ntiles = per_core // 128
    nsuper = ntiles // SUPER

    nc = bacc.Bacc("TRN2", target_bir_lowering=False, debug=False)

    d_fd = nc.dram_tensor("fd", [2, per_core], BF16, kind="ExternalInput")
    d_ph = nc.dram_tensor("ph", [1, per_core], BF16, kind="ExternalInput")
    d_mi = nc.dram_tensor("mi", [1, per_core], BF16, kind="ExternalInput")
    d_bfc = nc.dram_tensor("bfc", [128, BFC_COLS], BF16, kind="ExternalInput")
    d_f32c = nc.dram_tensor("f32c", [128, 2], F32, kind="ExternalInput")
    if apply_gb:
        d_gbc = nc.dram_tensor("g_bc", [128, 256], F32, kind="ExternalInput")
        d_bbc = nc.dram_tensor("b_bc", [128, 256], F32, kind="ExternalInput")
    d_out = nc.dram_tensor("out", [per_core, 256], F32, kind="ExternalOutput")

    with tile.TileContext(nc) as tc, ExitStack() as ctx:
        singles = ctx.enter_context(tc.tile_pool(name="singles", bufs=1))
        sb_oh = ctx.enter_context(tc.tile_pool(name="oh", bufs=2))
        sb_fdh = ctx.enter_context(tc.tile_pool(name="fdh", bufs=2))
        sb_small = ctx.enter_context(tc.tile_pool(name="small", bufs=3))
        sb_y = ctx.enter_context(tc.tile_pool(name="y", bufs=3))
        sb_yt = ctx.enter_context(tc.tile_pool(name="yt", bufs=3))
        sb_out = ctx.enter_context(tc.tile_pool(name="ostage", bufs=2))
        pp_bc = ctx.enter_context(tc.tile_pool(name="pbc", bufs=2, space="PSUM"))
        pp_fd = ctx.enter_context(tc.tile_pool(name="pfd", bufs=1, space="PSUM"))
        pp_h = ctx.enter_context(tc.tile_pool(name="ph_", bufs=2, space="PSUM"))
        pp_o = ctx.enter_context(tc.tile_pool(name="po", bufs=2, space="PSUM"))

        s_fd = singles.tile([2, per_core], BF16, tag="c_fd")
        nc.gpsimd.dma_start(out=s_fd[:], in_=d_fd[:])
        s_ph = singles.tile([1, per_core], BF16, tag="c_ph")
        nc.gpsimd.dma_start(out=s_ph[:], in_=d_ph[:])
        s_mi = singles.tile([1, per_core], BF16, tag="c_mi")
        nc.gpsimd.dma_start(out=s_mi[:], in_=d_mi[:])
        s_bfc = singles.tile([128, BFC_COLS], BF16, tag="c_bfc")
        nc.gpsimd.dma_start(out=s_bfc[:], in_=d_bfc[:])
        s_f32c = singles.tile([128, 2], F32, tag="c_f32c")
        nc.gpsimd.dma_start(out=s_f32c[:], in_=d_f32c[:])
        if apply_gb:
            s_gbc = singles.tile([128, 256], F32, tag="c_gbc")
            nc.gpsimd.dma_start(out=s_gbc[:], in_=d_gbc[:])
            s_bbc = singles.tile([128, 256], F32, tag="c_bbc")
            nc.gpsimd.dma_start(out=s_bbc[:], in_=d_bbc[:])
        s_eps = singles.tile([128, 1], F32, tag="eps")
        nc.vector.memset(s_eps, EPS)

        s_fdw = s_bfc[0:64, 0:256]
        s_phw = s_bfc[:, 256:512]
        s_miw = s_bfc[:, 512:768]
        s_w2a = s_bfc[:, 768:1024]
        s_w2b = s_bfc[:, 1024:1280]
        s_b2 = s_bfc[0:1, 1280:1536]
        s_ones = s_bfc[0:1, 1536:1664]
        s_w1 = s_bfc[0:2, 1664:1728]
        s_iota = s_f32c[:, 0:1]
        s_b1 = s_f32c[0:64, 1:2]

        out_r = d_out.ap().rearrange("(gi t p) c -> gi t p c", t=OUT_GROUP, p=128)

        ostage = None
        for st in range(nsuper):
            W = SUPER * 128
            sl = slice(st * W, (st + 1) * W)

            bc_ph = pp_bc.tile([128, W], F32, tag="bc")
            nc.tensor.matmul(bc_ph[:], s_ones, s_ph[:, sl], start=True, stop=True)
            oh_ph = sb_oh.tile([128, W], BF16, tag="ohp")
            nc.vector.tensor_scalar(
                out=oh_ph[:], in0=bc_ph[:], scalar1=s_iota, scalar2=None,
                op0=mybir.AluOpType.is_equal)
            bc_mi = pp_bc.tile([128, W], F32, tag="bc")
            nc.tensor.matmul(bc_mi[:], s_ones, s_mi[:, sl], start=True, stop=True)
            oh_mi = sb_oh.tile([128, W], BF16, tag="ohm")
            nc.vector.tensor_scalar(
                out=oh_mi[:], in0=bc_mi[:], scalar1=s_iota, scalar2=None,
                op0=mybir.AluOpType.is_equal)

            fdp = pp_fd.tile([64, W], F32, tag="fdp")
            nc.tensor.matmul(fdp[:], s_w1, s_fd[:, sl], start=True, stop=True)
            fdh = sb_fdh.tile([64, W], BF16, tag="fdh")
            nc.scalar.activation(
                out=fdh[:], in_=fdp[:], func=mybir.ActivationFunctionType.Relu,
                bias=s_b1, scale=1.0)

            for pair in range(SUPER // 2):
                hps = pp_h.tile([128, 2, 256], F32, tag="hps")
                for tt in range(2):
                    lo = (pair * 2 + tt) * 128
                    nc.tensor.matmul(hps[:, tt, :], fdh[:, lo:lo + 128], s_fdw,
                                     start=True, stop=False)
                    nc.tensor.matmul(hps[:, tt, :], oh_ph[:, lo:lo + 128], s_phw,
                                     start=False, stop=False)
                    nc.tensor.matmul(hps[:, tt, :], oh_mi[:, lo:lo + 128], s_miw,
                                     start=False, stop=True)
                stats = sb_small.tile([128, 2, 6], F32, tag="stats")
                mv = sb_small.tile([128, 2, 2], F32, tag="mv")
                for tt in range(2):
                    nc.vector.bn_stats(out=stats[:, tt, :], in_=hps[:, tt, :])
                    nc.vector.bn_aggr(out=mv[:, tt, :], in_=stats[:, tt, :])
                rstd = sb_small.tile([128, 2], F32, tag="rstd")
                nc.scalar.activation(
                    out=rstd[:], in_=mv[:, :, 1], func=mybir.ActivationFunctionType.Sqrt,
                    bias=s_eps[:], scale=1.0)
                nc.vector.reciprocal(out=rstd[:], in_=rstd[:])
                negmu = sb_small.tile([128, 2], F32, tag="negmu")
                nc.vector.tensor_scalar(
                    out=negmu[:], in0=mv[:, :, 0], scalar1=-1.0, scalar2=None,
                    op0=mybir.AluOpType.mult)
                nmr = sb_small.tile([128, 2], F32, tag="nmr")
                nc.vector.tensor_mul(out=nmr[:], in0=negmu[:], in1=rstd[:])

                for tt in range(2):
                    gt = st * SUPER + pair * 2 + tt
                    og = gt % OUT_GROUP
                    if og == 0:
                        ostage = sb_out.tile([128, OUT_GROUP, 256], F32, tag="ost")

                    y = sb_y.tile([128, 256], BF16, tag="y")
                    yn = sb_y.tile([128, 256], F32, tag="yn")
                    nc.scalar.activation(
                        out=yn[:], in_=hps[:, tt, :],
                        func=mybir.ActivationFunctionType.Identity,
                        bias=nmr[:, tt:tt + 1], scale=rstd[:, tt:tt + 1])
                    nc.vector.tensor_mul(out=yn[:], in0=yn[:], in1=s_gbc[:])
                    nc.vector.tensor_add(out=yn[:], in0=yn[:], in1=s_bbc[:])
                    nc.vector.tensor_scalar(
                        out=y[:], in0=yn[:], scalar1=0.0, scalar2=None,
                        op0=mybir.AluOpType.max)

                    yta = sb_yt.tile([128, 128], BF16, tag="yta")
                    ytb = sb_yt.tile([128, 128], BF16, tag="ytb")
                    nc.sync.dma_start_transpose(yta[:], y[:, 0:128])
                    nc.sync.dma_start_transpose(ytb[:], y[:, 128:256])

                    ops = pp_o.tile([128, 256], F32, tag="ops")
                    nc.tensor.matmul(ops[:], s_ones, s_b2, start=True, stop=False)
                    nc.tensor.matmul(ops[:], yta[:], s_w2a, start=False, stop=False)
                    nc.tensor.matmul(ops[:], ytb[:], s_w2b, start=False, stop=True)

                    if tt == 0:
                        nc.vector.tensor_copy(out=ostage[:, og, :], in_=ops[:])
                    else:
                        nc.scalar.copy(out=ostage[:, og, :], in_=ops[:])

                    if og == OUT_GROUP - 1:
                        gi = gt // OUT_GROUP
                        nc.gpsimd.dma_start(
                            out=out_r[gi].rearrange("t p c -> p t c"),
                            in_=ostage[:])
    nc.compile()
    return nc


def _prep_gb(inputs):
    """Host prep for the general (apply g/b on device) fallback path."""
    f0 = np.ascontiguousarray(inputs["f0"], dtype=np.float32)
    dur = np.ascontiguousarray(inputs["duration"], dtype=np.float32)
    phone = np.asarray(inputs["phone"])
    midi = np.asarray(inputs["midi"])

    w1f, b1f = np.asarray(inputs["f0_w1"], np.float32), np.asarray(inputs["f0_b1"], np.float32)
    w2f, b2f = np.asarray(inputs["f0_w2"], np.float32), np.asarray(inputs["f0_b2"], np.float32)
    w1d, b1d = np.asarray(inputs["dur_w1"], np.float32), np.asarray(inputs["dur_b1"], np.float32)
    w2d, b2d = np.asarray(inputs["dur_w2"], np.float32), np.asarray(inputs["dur_b2"], np.float32)
    pht = np.asarray(inputs["phone_table"], np.float32)
    mit = np.asarray(inputs["midi_table"], np.float32)
    W1, pb1 = np.asarray(inputs["proj_w1"], np.float32), np.asarray(inputs["proj_b1"], np.float32)
    ln_g, ln_b = np.asarray(inputs["ln_g"], np.float32), np.asarray(inputs["ln_b"], np.float32)
    W2, pb2 = np.asarray(inputs["proj_w2"], np.float32), np.asarray(inputs["proj_b2"], np.float32)

    W1_f0, W1_ph = W1[0:64], W1[64:192]
    W1_mi, W1_du = W1[192:256], W1[256:320]

    fdw = np.vstack([w2f @ W1_f0, w2d @ W1_du])
    bias_total = pb1 + b2f @ W1_f0 + b2d @ W1_du
    phw = np.zeros((128, 256), np.float32)
    phw[: pht.shape[0]] = pht @ W1_ph + bias_total
    miw = mit @ W1_mi

    bf = ml_dtypes.bfloat16
    bfc = np.zeros((128, BFC_COLS), np.float32)
    bfc[0:64, 0:256] = fdw
    bfc[:, 256:512] = phw
    bfc[:, 512:768] = miw
    bfc[:, 768:1024] = W2[0:128]
    bfc[:, 1024:1280] = W2[128:256]
    bfc[0, 1280:1536] = pb2
    bfc[0, 1536:1664] = 1.0
    bfc[0, 1664:1696] = w1f[0]
    bfc[1, 1696:1728] = w1d[0]
    f32c = np.zeros((128, 2), np.float32)
    f32c[:, 0] = np.arange(128)
    f32c[0:64, 1] = np.concatenate([b1f, b1d])

    consts = {
        "bfc": bfc.astype(bf), "f32c": f32c,
        "g_bc": np.broadcast_to(ln_g, (128, 256)).astype(np.float32).copy(),
        "b_bc": np.broadcast_to(ln_b, (128, 256)).astype(np.float32).copy(),
    }

    f0v = f0.reshape(NCORES, PER_CORE)
    durv = dur.reshape(NCORES, PER_CORE)
    phv = phone.reshape(NCORES, PER_CORE).astype(bf)
    miv = midi.reshape(NCORES, PER_CORE).astype(bf)

    in_maps = []
    for c in range(NCORES):
        m = dict(consts)
        m["fd"] = np.stack([f0v[c], durv[c]]).astype(bf)
        m["ph"] = phv[c].reshape(1, PER_CORE)
        m["mi"] = miv[c].reshape(1, PER_CORE)
        in_maps.append(m)
    return in_maps


def _prepare(inputs):
    ln_g = np.asarray(inputs["ln_g"], np.float32)
    ln_b = np.asarray(inputs["ln_b"], np.float32)
    fast = bool((ln_g > 0).all() and (ln_b == 0).all())
    if fast:
        add_pb2 = not bool((np.asarray(inputs["proj_b2"]) == 0).all())
        key = ("v2", add_pb2)
        if key not in _cache:
            _cache[key] = _build_program_v2(add_pb2)
        return _cache[key], _prep_v2(inputs)
    if "gb" not in _cache:
        _cache["gb"] = _build_program(True)
    return _cache["gb"], _prep_gb(inputs)


def kernel(**inputs) -> np.ndarray:
    nc, in_maps = _prepare(inputs)
    res = run_bass_kernel_spmd(nc, in_maps, core_ids=list(range(NCORES)))
    out = np.concatenate([r["out"] for r in res.results], axis=0)
    return out.reshape(B, T, COND).astype(np.float32)


# revision 29
# speedup vs baseline: 3.1397x; 2.6892x over previous
"""Trainium2 Bass kernel for nn_ConditioningEncoder.

Pipeline per position: f0/dur scalar MLPs + phone/midi embedding lookups
-> concat -> Linear(320,256) -> LayerNorm -> ReLU -> Linear(256,256).

Fast path (ln_g > 0, ln_b == 0 -- the actual problem instance):
- Host folds the small linears AND the LayerNorm mean-subtraction into
  the first projection: centering is linear, so W1' = W1 @ (I - J/256)
  makes the device's first-layer output h_c exactly zero-mean.  Then
  LN+ReLU reduces to  y = rstd * relu(h_c)  with rstd = 1/sqrt(mean
  (h_c^2)+eps), and g>0 folds into W2.
- Layer 1 runs COND-MAJOR: stationary weight blocks [K,128] stream the
  positions, producing h_c^T [2x128cond, W pos] straight into PSUM.
  This needs only 6 matmuls per 512-position supertile and -- key --
  makes h_c^T itself the stationary operand layer 2 needs, so the
  per-tile DMA transposes of the old design disappear entirely.
- Embedding gathers are one-hot matmuls; the one-hot operands are built
  by a GpSimd partition_broadcast + one DVE is_equal against iota
  (no PE broadcast matmuls).
- sum(h_c^2) via a ones-column matmul over both cond halves; rstd is
  broadcast back to all partitions with a K=1 ones-row matmul and
  applied to y in cond-major on GpSimd.
- Layer 2: lhsT = y^T slices (data stationary), rhs = W2 halves; the
  PSUM drain is a single DVE tensor_tensor add of the pre-broadcast
  output bias into the staged output buffer; 1MB output DMAs.

Fallback path (general ln_g/ln_b): previous-generation program (DMA
transposes + bn_stats), kept verbatim for correctness.
"""

import numpy as np
import ml_dtypes
from contextlib import ExitStack

import concourse.bass as bass
import concourse.mybir as mybir
import concourse.tile as tile
from concourse import bacc
from concourse.bass_utils import run_bass_kernel_spmd

BF16 = mybir.dt.bfloat16
F32 = mybir.dt.float32
NCORES = 8
B, T, COND = 16, 4096, 256
NPOS = B * T                     # 65536
PER_CORE = NPOS // NCORES        # 8192
NTILES = PER_CORE // 128         # 64 tiles of 128 positions
OUT_GROUP = 8                    # tiles per output DMA (1MB)
EPS = 1e-5
BFC_COLS = 1728

# ---- v2 (fast path) constants ----
W2SUP = 512                      # positions per supertile
NSUPER = PER_CORE // W2SUP       # 16
TPS = W2SUP // 128               # 4 tiles per supertile
V2_BCOLS = 1536

_cache = {}


def _build_program_v2(add_pb2: bool):
    per_core = PER_CORE
    W = W2SUP

    nc = bacc.Bacc("TRN2", target_bir_lowering=False, debug=False)

    d_fd = nc.dram_tensor("fd", [1, 2 * per_core], F32, kind="ExternalInput")
    d_phmi = nc.dram_tensor("phmi", [1, 2 * per_core], BF16, kind="ExternalInput")
    d_bfc = nc.dram_tensor("bfc", [128, V2_BCOLS], BF16, kind="ExternalInput")
    d_f32c = nc.dram_tensor("f32c", [128, 4], F32, kind="ExternalInput")
    d_pb2 = nc.dram_tensor("pb2bc", [128, 256], F32, kind="ExternalInput")
    d_out = nc.dram_tensor("out", [per_core, 256], F32, kind="ExternalOutput")

    with tile.TileContext(nc) as tc, ExitStack() as ctx:
        singles = ctx.enter_context(tc.tile_pool(name="singles", bufs=1))
        p_bc = ctx.enter_context(tc.tile_pool(name="bc", bufs=3))
        p_oh = ctx.enter_context(tc.tile_pool(name="oh", bufs=3))
        p_fd = ctx.enter_context(tc.tile_pool(name="fd", bufs=3))
        p_h = ctx.enter_context(tc.tile_pool(name="hsb", bufs=3))
        p_y = ctx.enter_context(tc.tile_pool(name="y", bufs=3))
        p_sm = ctx.enter_context(tc.tile_pool(name="sm", bufs=3))
        p_ost = ctx.enter_context(tc.tile_pool(name="ost", bufs=2))
        pp_h = ctx.enter_context(tc.tile_pool(name="ph_", bufs=2, space="PSUM"))
        pp_v = ctx.enter_context(tc.tile_pool(name="pv", bufs=2, space="PSUM"))
        pp_o = ctx.enter_context(tc.tile_pool(name="po", bufs=2, space="PSUM"))

        # ---- load constants (inputs stream in broadcast per-super) ----
        s_bfc = singles.tile([128, V2_BCOLS], BF16, tag="c_bfc")
        nc.gpsimd.dma_start(out=s_bfc[:], in_=d_bfc[:])
        s_f32c = singles.tile([128, 4], F32, tag="c_f32c")
        nc.gpsimd.dma_start(out=s_f32c[:], in_=d_f32c[:])
        if add_pb2:
            s_pb2 = singles.tile([128, 256], F32, tag="c_pb2")
            nc.gpsimd.dma_start(out=s_pb2[:], in_=d_pb2[:])

        s_fdw = s_bfc[0:64, 0:256]       # [64, 2x128] cond-major fd weights
        s_phw = s_bfc[:, 256:512]        # [128, 2x128]
        s_miw = s_bfc[:, 512:768]
        s_w2a = s_bfc[:, 768:1024]       # W2 rows 0:128 -> [128, 256]
        s_w2b = s_bfc[:, 1024:1280]
        s_ones_blk = s_bfc[:, 1280:1408]     # [128, 128] of 1.0
        s_iota = s_f32c[:, 0:1]
        s_w1 = s_f32c[0:64, 1:2]
        s_b1 = s_f32c[0:64, 2:3]
        s_eps = s_f32c[:, 3:4]               # [128, 1] of EPS

        phmi_r = d_phmi.ap().rearrange("p (c n) -> p c n", c=2)  # [1, 2, per_core]
        fd_r = d_fd.ap().rearrange("p (c n) -> p c n", c=2)      # [1, 2, per_core]
        out_r = d_out.ap().rearrange("(gi t p) c -> gi t p c", t=OUT_GROUP, p=128)

        # Software-pipelined emission: engines execute their queues in
        # order, so the front stage of supertile s+2 is emitted before the
        # stats/layer-2 stages of supertile s.  This keeps the PE streaming
        # layer-1 matmuls instead of head-of-line blocking on the
        # h -> h^2 -> var -> rstd -> y_n chain of the current supertile.
        sup = {}      # per-super live tiles
        state = {"ostage": None}

        def front(s):
            sl = slice(s * W, (s + 1) * W)
            # one-hot build: DMA-broadcast ph|mi to all partitions
            # (stride-0 source), compare against per-partition iota
            bc = p_bc.tile([128, 2, W], BF16, tag="bc")
            nc.sync.dma_start(out=bc[:], in_=phmi_r[:, :, sl].broadcast_to([128, 2, W]))
            oh = p_oh.tile([128, 2, W], BF16, tag="oh")
            nc.vector.tensor_scalar(
                out=oh[:], in0=bc[:], scalar1=s_iota, scalar2=None,
                op0=mybir.AluOpType.is_equal)
            # f0/dur hidden: broadcast raw scalars, then per-partition w1/b1
            fdbc = p_fd.tile([64, 2, W], F32, tag="fdbc")
            nc.sync.dma_start(out=fdbc[:], in_=fd_r[:, :, sl].broadcast_to([64, 2, W]))
            fdh = p_fd.tile([64, W], BF16, tag="fdh")
            nc.scalar.activation(
                out=fdh[0:32, :], in_=fdbc[0:32, 0, :],
                func=mybir.ActivationFunctionType.Relu,
                bias=s_b1[0:32, :], scale=s_w1[0:32, :])
            nc.scalar.activation(
                out=fdh[32:64, :], in_=fdbc[32:64, 1, :],
                func=mybir.ActivationFunctionType.Relu,
                bias=s_b1[32:64, :], scale=s_w1[32:64, :])
            # layer 1, cond-major: h_c^T [128, 2(half), W]
            hps = pp_h.tile([128, 2, W], F32, tag="hps")
            for hf in range(2):
                cs = slice(hf * 128, (hf + 1) * 128)
                nc.tensor.matmul(hps[:, hf, :], s_fdw[:, cs], fdh[:],
                                 start=True, stop=False)
                nc.tensor.matmul(hps[:, hf, :], s_phw[:, cs], oh[:, 0, :],
                                 start=False, stop=False)
                nc.tensor.matmul(hps[:, hf, :], s_miw[:, cs], oh[:, 1, :],
                                 start=False, stop=True)
            sup[s] = {"hps": hps}

        def mid(s):
            hps = sup[s]["hps"]
            # square straight from PSUM (no staging copy)
            hsq = p_h.tile([128, 2, W], BF16, tag="hsq")
            nc.scalar.activation(
                out=hsq[:], in_=hps[:], func=mybir.ActivationFunctionType.Square)
            # sum h^2 over both cond halves with an all-ones stationary
            # block: every output partition gets the same row of sums
            var_ps = pp_v.tile([128, W], F32, tag="var")
            nc.tensor.matmul(var_ps[:], s_ones_blk, hsq[:, 0, :],
                             start=True, stop=False)
            nc.tensor.matmul(var_ps[:], s_ones_blk, hsq[:, 1, :],
                             start=False, stop=True)
            # rstd = 1/sqrt(sumsq/256 + eps), full-width on all partitions
            sd = p_sm.tile([128, W], F32, tag="sd")
            nc.scalar.activation(
                out=sd[:], in_=var_ps[:], func=mybir.ActivationFunctionType.Sqrt,
                bias=s_eps, scale=1.0 / COND)
            rbc = p_sm.tile([128, W], F32, tag="rbcsb")
            nc.vector.reciprocal_approx_fast(out=rbc[:], in_=sd[:])
            # y_n = relu(h) * rstd fused in one DVE op straight from PSUM
            yn = p_y.tile([128, 2, W], BF16, tag="yn")
            nc.vector.scalar_tensor_tensor(
                out=yn[:], in0=hps[:], scalar=0.0,
                in1=rbc.rearrange("p (c n) -> p c n", c=1).broadcast_to([128, 2, W]),
                op0=mybir.AluOpType.max, op1=mybir.AluOpType.mult)
            sup[s]["yn"] = yn

        def back(s):
            yn = sup.pop(s)["yn"]
            # layer 2 per 128-pos tile: lhsT = y_n^T slices, rhs = W2 halves
            for t in range(TPS):
                gt = s * TPS + t
                og = gt % OUT_GROUP
                if og == 0:
                    state["ostage"] = p_ost.tile(
                        [128, OUT_GROUP, 256], F32, tag="ost", name="ostage")
                ostage = state["ostage"]
                ch = slice(t * 128, (t + 1) * 128)
                ops = pp_o.tile([128, 256], F32, tag="ops")
                nc.tensor.matmul(ops[:], yn[:, 0, ch], s_w2a,
                                 start=True, stop=False)
                nc.tensor.matmul(ops[:], yn[:, 1, ch], s_w2b,
                                 start=False, stop=True)
                if add_pb2:
                    # drain + output bias in one DVE op
                    nc.vector.tensor_add(out=ostage[:, og, :], in0=ops[:], in1=s_pb2[:])
                elif t % 2 == 0:
                    nc.scalar.copy(out=ostage[:, og, :], in_=ops[:])
                else:
                    nc.vector.tensor_copy(out=ostage[:, og, :], in_=ops[:])
                if og == OUT_GROUP - 1:
                    gi = gt // OUT_GROUP
                    nc.gpsimd.dma_start(
                        out=out_r[gi].rearrange("t p c -> p t c"),
                        in_=ostage[:])

        for it in range(NSUPER + 2):
            if it < NSUPER:
                front(it)
            if 1 <= it <= NSUPER:
                mid(it - 1)
            if it >= 2:
                back(it - 2)
    nc.compile()
    return nc


def _prep_v2(inputs):
    f0 = np.ascontiguousarray(inputs["f0"], dtype=np.float32)
    dur = np.ascontiguousarray(inputs["duration"], dtype=np.float32)
    phone = np.asarray(inputs["phone"])
    midi = np.asarray(inputs["midi"])

    w1f, b1f = np.asarray(inputs["f0_w1"], np.float32), np.asarray(inputs["f0_b1"], np.float32)
    w2f, b2f = np.asarray(inputs["f0_w2"], np.float32), np.asarray(inputs["f0_b2"], np.float32)
    w1d, b1d = np.asarray(inputs["dur_w1"], np.float32), np.asarray(inputs["dur_b1"], np.float32)
    w2d, b2d = np.asarray(inputs["dur_w2"], np.float32), np.asarray(inputs["dur_b2"], np.float32)
    pht = np.asarray(inputs["phone_table"], np.float32)
    mit = np.asarray(inputs["midi_table"], np.float32)
    W1, pb1 = np.asarray(inputs["proj_w1"], np.float32), np.asarray(inputs["proj_b1"], np.float32)
    ln_g = np.asarray(inputs["ln_g"], np.float32)
    W2, pb2 = np.asarray(inputs["proj_w2"], np.float32), np.asarray(inputs["proj_b2"], np.float32)

    W1_f0, W1_ph = W1[0:64], W1[64:192]
    W1_mi, W1_du = W1[192:256], W1[256:320]

    # fold LN mean-centering (linear!) into the first projection
    C = np.eye(COND, dtype=np.float64) - 1.0 / COND
    fdw = (np.vstack([w2f @ W1_f0, w2d @ W1_du]) @ C).astype(np.float32)
    bias_total = ((pb1 + b2f @ W1_f0 + b2d @ W1_du) @ C).astype(np.float32)
    phw = np.zeros((128, COND), np.float32)
    phw[: pht.shape[0]] = ((pht @ W1_ph) @ C).astype(np.float32) + bias_total
    miw = ((mit @ W1_mi) @ C).astype(np.float32)

    W2e = ln_g[:, None] * W2

    bf = ml_dtypes.bfloat16
    bfc = np.zeros((128, V2_BCOLS), np.float32)
    bfc[0:64, 0:256] = fdw
    bfc[:, 256:512] = phw
    bfc[:, 512:768] = miw
    bfc[:, 768:1024] = W2e[0:128]
    bfc[:, 1024:1280] = W2e[128:256]
    bfc[:, 1280:1408] = 1.0
    f32c = np.zeros((128, 4), np.float32)
    f32c[:, 0] = np.arange(128)
    f32c[0:32, 1] = w1f[0]
    f32c[32:64, 1] = w1d[0]
    f32c[0:64, 2] = np.concatenate([b1f, b1d])
    f32c[:, 3] = EPS
    pb2bc = np.broadcast_to(pb2, (128, 256)).astype(np.float32).copy()

    consts = {"bfc": bfc.astype(bf), "f32c": f32c, "pb2bc": pb2bc}

    f0v = f0.reshape(NCORES, PER_CORE)
    durv = dur.reshape(NCORES, PER_CORE)
    phv = phone.reshape(NCORES, PER_CORE).astype(bf)
    miv = midi.reshape(NCORES, PER_CORE).astype(bf)

    in_maps = []
    for c in range(NCORES):
        m = dict(consts)
        m["fd"] = np.concatenate([f0v[c], durv[c]]).reshape(1, 2 * PER_CORE)
        m["phmi"] = np.concatenate([phv[c], miv[c]]).reshape(1, 2 * PER_CORE)
        in_maps.append(m)
    return in_maps


# ======================= fallback (general ln_g/ln_b) =======================

SUPER = 4
BFC_COLS = 1728


def _build_program(apply_gb: bool):
    per_core = PER_CORE
    ntiles = per_core // 128
    nsuper = ntiles // SUPER

    nc = bacc.Bacc("TRN2", target_bir_lowering=False, debug=False)

    d_fd = nc.dram_tensor("fd", [2, per_core], BF16, kind="ExternalInput")
    d_ph = nc.dram_tensor("ph", [1, per_core], BF16, kind="ExternalInput")
    d_mi = nc.dram_tensor("mi", [1, per_core], BF16, kind="ExternalInput")
    d_bfc = nc.dram_tensor("bfc", [128, BFC_COLS], BF16, kind="ExternalInput")
    d_f32c = nc.dram_tensor("f32c", [128, 2], F32, kind="ExternalInput")
    if apply_gb:
        d_gbc = nc.dram_tensor("g_bc", [128, 256], F32, kind="ExternalInput")
        d_bbc = nc.dram_tensor("b_bc", [128, 256], F32, kind="ExternalInput")
    d_out = nc.dram_tensor("out", [per_core, 256], F32, kind="ExternalOutput")

    with tile.TileContext(nc) as tc, ExitStack() as ctx:
        singles = ctx.enter_context(tc.tile_pool(name="singles", bufs=1))
        sb_oh = ctx.enter_context(tc.tile_pool(name="oh", bufs=2))
        sb_fdh = ctx.enter_context(tc.tile_pool(name="fdh", bufs=2))
        sb_small = ctx.enter_context(tc.tile_pool(name="small", bufs=3))
        sb_y = ctx.enter_context(tc.tile_pool(name="y", bufs=3))
        sb_yt = ctx.enter_context(tc.tile_pool(name="yt", bufs=3))
        sb_out = ctx.enter_context(tc.tile_pool(name="ostage", bufs=2))
        pp_bc = ctx.enter_context(tc.tile_pool(name="pbc", bufs=2, space="PSUM"))
        pp_fd = ctx.enter_context(tc.tile_pool(name="pfd", bufs=1, space="PSUM"))
        pp_h = ctx.enter_context(tc.tile_pool(name="ph_", bufs=2, space="PSUM"))
        pp_o = ctx.enter_context(tc.tile_pool(name="po", bufs=2, space="PSUM"))

        s_fd = singles.tile([2, per_core], BF16, tag="c_fd")
        nc.gpsimd.dma_start(out=s_fd[:], in_=d_fd[:])
        s_ph = singles.tile([1, per_core], BF16, tag="c_ph")
        nc.gpsimd.dma_start(out=s_ph[:], in_=d_ph[:])
        s_mi = singles.tile([1, per_core], BF16, tag="c_mi")
        nc.gpsimd.dma_start(out=s_mi[:], in_=d_mi[:])
        s_bfc = singles.tile([128, BFC_COLS], BF16, tag="c_bfc")
        nc.gpsimd.dma_start(out=s_bfc[:], in_=d_bfc[:])
        s_f32c = singles.tile([128, 2], F32, tag="c_f32c")
        nc.gpsimd.dma_start(out=s_f32c[:], in_=d_f32c[:])
        if apply_gb:
            s_gbc = singles.tile([128, 256], F32, tag="c_gbc")
            nc.gpsimd.dma_start(out=s_gbc[:], in_=d_gbc[:])
            s_bbc = singles.tile([128, 256], F32, tag="c_bbc")
            nc.gpsimd.dma_start(out=s_bbc[:], in_=d_bbc[:])
        s_eps = singles.tile([128, 1], F32, tag="eps")
        nc.vector.memset(s_eps, EPS)

        s_fdw = s_bfc[0:64, 0:256]
        s_phw = s_bfc[:, 256:512]
        s_miw = s_bfc[:, 512:768]
        s_w2a = s_bfc[:, 768:1024]
        s_w2b = s_bfc[:, 1024:1280]
        s_b2 = s_bfc[0:1, 1280:1536]
        s_ones = s_bfc[0:1, 1536:1664]
        s_w1 = s_bfc[0:2, 1664:1728]
        s_iota = s_f32c[:, 0:1]
        s_b1 = s_f32c[0:64, 1:2]

        out_r = d_out.ap().rearrange("(gi t p) c -> gi t p c", t=OUT_GROUP, p=128)

        ostage = None
        for st in range(nsuper):
            W = SUPER * 128
            sl = slice(st * W, (st + 1) * W)

            bc_ph = pp_bc.tile([128, W], F32, tag="bc")
            nc.tensor.matmul(bc_ph[:], s_ones, s_ph[:, sl], start=True, stop=True)
            oh_ph = sb_oh.tile([128, W], BF16, tag="ohp")
            nc.vector.tensor_scalar(
                out=oh_ph[:], in0=bc_ph[:], scalar1=s_iota, scalar2=None,
                op0=mybir.AluOpType.is_equal)
            bc_mi = pp_bc.tile([128, W], F32, tag="bc")
            nc.tensor.matmul(bc_mi[:], s_ones, s_mi[:, sl], start=True, stop=True)
            oh_mi = sb_oh.tile([128, W], BF16, tag="ohm")
            nc.vector.tensor_scalar(
                out=oh_mi[:], in0=bc_mi[:], scalar1=s_iota, scalar2=None,
                op0=mybir.AluOpType.is_equal)

            fdp = pp_fd.tile([64, W], F32, tag="fdp")
            nc.tensor.matmul(fdp[:], s_w1, s_fd[:, sl], start=True, stop=True)
            fdh = sb_fdh.tile([64, W], BF16, tag="fdh")
            nc.scalar.activation(
                out=fdh[:], in_=fdp[:], func=mybir.ActivationFunctionType.Relu,
                bias=s_b1, scale=1.0)

            for pair in range(SUPER // 2):
                hps = pp_h.tile([128, 2, 256], F32, tag="hps")
                for tt in range(2):
                    lo = (pair * 2 + tt) * 128
                    nc.tensor.matmul(hps[:, tt, :], fdh[:, lo:lo + 128], s_fdw,
                                     start=True, stop=False)
                    nc.tensor.matmul(hps[:, tt, :], oh_ph[:, lo:lo + 128], s_phw,
                                     start=False, stop=False)
                    nc.tensor.matmul(hps[:, tt, :], oh_mi[:, lo:lo + 128], s_miw,
                                     start=False, stop=True)
                stats = sb_small.tile([128, 2, 6], F32, tag="stats")
                mv = sb_small.tile([128, 2, 2], F32, tag="mv")
                for tt in range(2):
                    nc.vector.bn_stats(out=stats[:, tt, :], in_=hps[:, tt, :])
                    nc.vector.bn_aggr(out=mv[:, tt, :], in_=stats[:, tt, :])
                rstd = sb_small.tile([128, 2], F32, tag="rstd")
                nc.scalar.activation(
                    out=rstd[:], in_=mv[:, :, 1], func=mybir.ActivationFunctionType.Sqrt,
                    bias=s_eps[:], scale=1.0)
                nc.vector.reciprocal(out=rstd[:], in_=rstd[:])
                negmu = sb_small.tile([128, 2], F32, tag="negmu")
                nc.vector.tensor_scalar(
                    out=negmu[:], in0=mv[:, :, 0], scalar1=-1.0, scalar2=None,
                    op0=mybir.AluOpType.mult)
                nmr = sb_small.tile([128, 2], F32, tag="nmr")
                nc.vector.tensor_mul(out=nmr[:], in0=negmu[:], in1=rstd[:])

                for tt in range(2):
                    gt = st * SUPER + pair * 2 + tt
                    og = gt % OUT_GROUP
                    if og == 0:
                        ostage = sb_out.tile([128, OUT_GROUP, 256], F32, tag="ost")

                    y = sb_y.tile([128, 256], BF16, tag="y")
                    yn = sb_y.tile([128, 256], F32, tag="yn")
                    nc.scalar.activation(
                        out=yn[:], in_=hps[:, tt, :],
                        func=mybir.ActivationFunctionType.Identity,
                        bias=nmr[:, tt:tt + 1], scale=rstd[:, tt:tt + 1])
                    nc.vector.tensor_mul(out=yn[:], in0=yn[:], in1=s_gbc[:])
                    nc.vector.tensor_add(out=yn[:], in0=yn[:], in1=s_bbc[:])
                    nc.vector.tensor_scalar(
                        out=y[:], in0=yn[:], scalar1=0.0, scalar2=None,
                        op0=mybir.AluOpType.max)

                    yta = sb_yt.tile([128, 128], BF16, tag="yta")
                    ytb = sb_yt.tile([128, 128], BF16, tag="ytb")
                    nc.sync.dma_start_transpose(yta[:], y[:, 0:128])
                    nc.sync.dma_start_transpose(ytb[:], y[:, 128:256])

                    ops = pp_o.tile([128, 256], F32, tag="ops")
                    nc.tensor.matmul(ops[:], s_ones, s_b2, start=True, stop=False)
                    nc.tensor.matmul(ops[:], yta[:], s_w2a, start=False, stop=False)
                    nc.tensor.matmul(ops[:], ytb[:], s_w2b, start=False, stop=True)

                    if tt == 0:
                        nc.vector.tensor_copy(out=ostage[:, og, :], in_=ops[:])
                    else:
                        nc.scalar.copy(out=ostage[:, og, :], in_=ops[:])

                    if og == OUT_GROUP - 1:
                        gi = gt // OUT_GROUP
                        nc.gpsimd.dma_start(
                            out=out_r[gi].rearrange("t p c -> p t c"),
                            in_=ostage[:])
    nc.compile()
    return nc


def _prep_gb(inputs):
    """Host prep for the general (apply g/b on device) fallback path."""
    f0 = np.ascontiguousarray(inputs["f0"], dtype=np.float32)
    dur = np.ascontiguousarray(inputs["duration"], dtype=np.float32)
    phone = np.asarray(inputs["phone"])
    midi = np.asarray(inputs["midi"])

    w1f, b1f = np.asarray(inputs["f0_w1"], np.float32), np.asarray(inputs["f0_b1"], np.float32)
    w2f, b2f = np.asarray(inputs["f0_w2"], np.float32), np.asarray(inputs["f0_b2"], np.float32)
    w1d, b1d = np.asarray(inputs["dur_w1"], np.float32), np.asarray(inputs["dur_b1"], np.float32)
    w2d, b2d = np.asarray(inputs["dur_w2"], np.float32), np.asarray(inputs["dur_b2"], np.float32)
    pht = np.asarray(inputs["phone_table"], np.float32)
    mit = np.asarray(inputs["midi_table"], np.float32)
    W1, pb1 = np.asarray(inputs["proj_w1"], np.float32), np.asarray(inputs["proj_b1"], np.float32)
    ln_g, ln_b = np.asarray(inputs["ln_g"], np.float32), np.asarray(inputs["ln_b"], np.float32)
    W2, pb2 = np.asarray(inputs["proj_w2"], np.float32), np.asarray(inputs["proj_b2"], np.float32)

    W1_f0, W1_ph = W1[0:64], W1[64:192]
    W1_mi, W1_du = W1[192:256], W1[256:320]

    fdw = np.vstack([w2f @ W1_f0, w2d @ W1_du])
    bias_total = pb1 + b2f @ W1_f0 + b2d @ W1_du
    phw = np.zeros((128, 256), np.float32)
    phw[: pht.shape[0]] = pht @ W1_ph + bias_total
    miw = mit @ W1_mi

    bf = ml_dtypes.bfloat16
    bfc = np.zeros((128, BFC_COLS), np.float32)
    bfc[0:64, 0:256] = fdw
    bfc[:, 256:512] = phw
    bfc[:, 512:768] = miw
    bfc[:, 768:1024] = W2[0:128]
    bfc[:, 1024:1280] = W2[128:256]
    bfc[0, 1280:1536] = pb2
    bfc[0, 1536:1664] = 1.0
    bfc[0, 1664:1696] = w1f[0]
    bfc[1, 1696:1728] = w1d[0]
    f32c = np.zeros((128, 2), np.float32)
    f32c[:, 0] = np.arange(128)
    f32c[0:64, 1] = np.concatenate([b1f, b1d])

    consts = {
        "bfc": bfc.astype(bf), "f32c": f32c,
        "g_bc": np.broadcast_to(ln_g, (128, 256)).astype(np.float32).copy(),
        "b_bc": np.broadcast_to(ln_b, (128, 256)).astype(np.float32).copy(),
    }

    f0v = f0.reshape(NCORES, PER_CORE)
    durv = dur.reshape(NCORES, PER_CORE)
    phv = phone.reshape(NCORES, PER_CORE).astype(bf)
    miv = midi.reshape(NCORES, PER_CORE).astype(bf)

    in_maps = []
    for c in range(NCORES):
        m = dict(consts)
        m["fd"] = np.stack([f0v[c], durv[c]]).astype(bf)
        m["ph"] = phv[c].reshape(1, PER_CORE)
        m["mi"] = miv[c].reshape(1, PER_CORE)
        in_maps.append(m)
    return in_maps


def _prepare(inputs):
    ln_g = np.asarray(inputs["ln_g"], np.float32)
    ln_b = np.asarray(inputs["ln_b"], np.float32)
    fast = bool((ln_g > 0).all() and (ln_b == 0).all())
    if fast:
        add_pb2 = not bool((np.asarray(inputs["proj_b2"]) == 0).all())
        key = ("v2", add_pb2)
        if key not in _cache:
            _cache[key] = _build_program_v2(add_pb2)
        return _cache[key], _prep_v2(inputs)
    if "gb" not in _cache:
        _cache["gb"] = _build_program(True)
    return _cache["gb"], _prep_gb(inputs)


def kernel(**inputs) -> np.ndarray:
    nc, in_maps = _prepare(inputs)
    res = run_bass_kernel_spmd(nc, in_maps, core_ids=list(range(NCORES)))
    out = np.concatenate([r["out"] for r in res.results], axis=0)
    return out.reshape(B, T, COND).astype(np.float32)
